# revision 5
# baseline (speedup 1.0000x reference)
"""Trainium2 Bass kernel for BDH recurrent (chunked linear) attention.

Problem shapes (hardcoded): Q_raw [2,16,2048,256] f32, V_raw [2,2048,1024] f32,
out [2,16,2048,1024] f32.  8 NeuronCores, data+head parallel: each core owns
4 (batch, head) pairs; V is shared across the 4 heads of a core's batch.

Math (reference semantics), per (b,h), chunks of 128:
  QR = rope(Q); KR = QR
  out_c = q_c @ state_{<c} + (q_c q_c^T  * strict_tril) v_c
  state += q_c^T v_c

v2 design:
  * fp16 everywhere 16-bit (same PE/DVE speed as bf16, ~8x the accuracy),
    which buys error budget for:
  * fp8(e4m3) DoubleRow PV matmuls: per superchunk of SUP=2 chunks, the
    transposed score blocks G_j (j = the 2 chunks) are evacuated into one
    [128, 2, 256] fp8 pair tile; the PV for chunk i is then ONE DoubleRow
    matmul per D-half contracting 256 rows (both j chunks) at 2x rate.
    The pair row of the later chunk j1 is zero in its leading 128 cols
    (evac reads 128 stale PSUM cols x a zero mask), preserving causality.
  * SUP=2 minimizes total matmul work (intra-superchunk O(S^2) term vs
    the per-chunk state/inter term).
  * state/inter matmuls stay fp16 (fp8 there fails the 2e-2 gate).
  * PSUM-resident fp32 state, cast to fp16 SBUF between superchunks.
  * evacuation/cast copies spread over scalar+gpsimd (knobs below); DVE
    keeps rope + masked G evacuation.

RoPE is computed twice, in the natural [t, n] layout (for the state update's
lhsT) and in the transposed [n, t] layout (for the G lhsT and the inter
lhsT) — the host supplies Q in both layouts (pair-deinterleaved:
(evens | odds), so the rotation is plane-wise multiply/adds with no
interleave shuffles), and rope runs in-place on the loaded tiles.
All DRAM layouts are partition-major so every DMA is 128 contiguous
descriptors; the output is written partition-major and un-permuted on host.
"""

import numpy as np
import ml_dtypes

import concourse.mybir as mybir
import concourse.tile as tile
from concourse import bacc
from concourse.bass import ds
from concourse.bass_utils import run_bass_kernel_spmd
from concourse.masks import make_identity

B, NH, T, N, D = 2, 16, 2048, 256, 1024
P = 128          # partition / chunk size
NCH = T // P     # 16 chunks
SUP = 2          # chunks per superchunk
NSUP = NCH // SUP
HPC = 4          # (b,h) pairs per core
NCORES = 8
THETA = 2.0 ** 16
TWO_PI = 2.0 * np.pi

f16 = mybir.dt.float16
f8 = mybir.dt.float8e4
f32 = mybir.dt.float32
f16_np = np.float16
f8_np = ml_dtypes.float8_e4m3  # TRN-style e4m3 (max normal 240)

mult = mybir.AluOpType.mult
add_op = mybir.AluOpType.add
sub_op = mybir.AluOpType.subtract
DR = mybir.MatmulPerfMode.DoubleRow

# engine assignment knobs (tuned from profiles).
# NB: gpsimd has NO PSUM port (walrus birverifier rejects it) and its
# tensor_tensor contends with DVE's shared SBUF port -> PSUM evacuation
# copies go on scalar/vector only; DVE keeps rope + masked G evac.
STATE_CAST_ENG = ("v", "s")   # by m-plane
OUT_EVAC_ENG = ("s", "s")     # by chunk parity


def _eng(nc, c):
    return {"g": nc.gpsimd, "v": nc.vector, "s": nc.scalar}[c]


def _copy(nc, c, out, in_):
    if c == "s":
        nc.scalar.copy(out, in_)
    else:
        _eng(nc, c).tensor_copy(out, in_)


def _emit_body(nc, tc, qn, qt, v, v8, cn, sn, ct, st, mskT, out):
    """Tile program for one core: 4 (b,h) pairs, full scan each."""
    with (
        tc.tile_pool(name="const", bufs=1) as constp,
        tc.tile_pool(name="qpool", bufs=2) as qpool,
        tc.tile_pool(name="work", bufs=4) as work,
        tc.tile_pool(name="tmppool", bufs=1) as tmpp,
        tc.tile_pool(name="outbuf", bufs=1) as outp,
        tc.tile_pool(name="statesb", bufs=2) as statep,
        tc.tile_pool(name="ps_state", bufs=1, space="PSUM") as ps_state,
        tc.tile_pool(name="ps_out", bufs=2, space="PSUM") as ps_out,
        tc.tile_pool(name="ps_g", bufs=2, space="PSUM") as ps_g,
    ):
        # resident constants (all DRAM layouts partition-major/contiguous).
        # Load order matters for the startup ramp: the first-superchunk
        # slices of the transposed-rope tables gate the first rope ops ->
        # first G matmuls, so they go first; v8's first slice feeds the
        # first PV.
        S0 = SUP * P  # first-slice width in the t dimension
        ct_sb = constp.tile([P, T], f16)
        st_sb = constp.tile([P, T], f16)
        nc.sync.dma_start(ct_sb[:, ds(0, S0)], ct[:, ds(0, S0)])
        nc.sync.dma_start(st_sb[:, ds(0, S0)], st[:, ds(0, S0)])
        msk_sb = constp.tile([P, 2, SUP * P], f16)
        nc.sync.dma_start(msk_sb[:], mskT[:, :, :])
        v8_sb = constp.tile([P, NCH, D], f8)
        nc.sync.dma_start(v8_sb[:, :SUP], v8[:, :SUP, :])
        v_sb = constp.tile([P, NCH, D], f16)
        nc.sync.dma_start(v_sb[:, :SUP], v[:, :SUP, :])
        nc.sync.dma_start(ct_sb[:, ds(S0, T - S0)], ct[:, ds(S0, T - S0)])
        nc.sync.dma_start(st_sb[:, ds(S0, T - S0)], st[:, ds(S0, T - S0)])
        ident = constp.tile([P, P], f16)
        make_identity(nc, ident)
        cn_sb = constp.tile([P, NCH, P], f16)
        nc.sync.dma_start(cn_sb[:], cn[:, :, :])
        sn_sb = constp.tile([P, NCH, P], f16)
        nc.sync.dma_start(sn_sb[:], sn[:, :, :])
        nc.sync.dma_start(v8_sb[:, SUP:], v8[:, SUP:, :])
        nc.sync.dma_start(v_sb[:, SUP : NCH // 2], v[:, SUP : NCH // 2, :])
        nc.sync.dma_start(v_sb[:, NCH // 2 :], v[:, NCH // 2 :, :])

        for bh in range(HPC):
            # q tiles; rope runs in-place so qt_sb becomes qrT and qn_sb
            # becomes qr (natural layout).
            qt_sb = qpool.tile([P, 2, T], f16, tag="qt")
            if bh == 0:
                for m in range(2):
                    nc.scalar.dma_start(qt_sb[:, m, ds(0, S0)], qt[bh, m, :, ds(0, S0)])
                for m in range(2):
                    nc.scalar.dma_start(
                        qt_sb[:, m, ds(S0, T - S0)], qt[bh, m, :, ds(S0, T - S0)]
                    )
            else:
                nc.scalar.dma_start(qt_sb[:, 0], qt[bh, 0])
                nc.scalar.dma_start(qt_sb[:, 1], qt[bh, 1])
            qn_sb = qpool.tile([P, 2, NCH, P], f16, tag="qn")
            nc.scalar.dma_start(qn_sb[:], qn[bh])

            # RoPE, transposed layout [n', t], in-place on qt_sb — emitted
            # FIRST: it gates the G matmuls, and the DVE queue is strict
            # FIFO.  qr_e = qe*c - qo*s ; qr_o = qo*c + qe*s, with the tmp
            # products computed before the in-place overwrite of qe/qo.
            tmp2 = tmpp.tile([P, T], f16, tag="ropetmp2")
            tmp3 = tmpp.tile([P, T], f16, tag="ropetmp3")

            def emit_t_rope(c0, w):
                sl = ds(c0, w)
                qte, qto = qt_sb[:, 0, sl], qt_sb[:, 1, sl]
                t2, t3 = tmp2[:, sl], tmp3[:, sl]
                nc.vector.tensor_tensor(t2, qto, st_sb[:, sl], mult)
                nc.vector.tensor_tensor(t3, qte, st_sb[:, sl], mult)
                nc.vector.tensor_tensor(qte, qte, ct_sb[:, sl], mult)
                nc.vector.tensor_tensor(qto, qto, ct_sb[:, sl], mult)
                nc.vector.tensor_tensor(qte, qte, t2, sub_op)
                nc.vector.tensor_tensor(qto, qto, t3, add_op)

            if bh == 0:
                # split so the first superchunk's G matmuls start early
                emit_t_rope(0, S0)
                emit_t_rope(S0, T - S0)
            else:
                emit_t_rope(0, T)

            # RoPE, natural layout, in-place on qn_sb (planes (evens|odds)).
            # Emitted lazily (at s==0, after the first G evacuations): qr
            # only gates the state-update matmuls.
            def emit_nat_rope(c0=0):
                tmp0 = tmpp.tile([P, NCH, P], f16, tag="ropetmp0")
                tmp1 = tmpp.tile([P, NCH, P], f16, tag="ropetmp1")
                qe, qo = qn_sb[:, 0, c0:], qn_sb[:, 1, c0:]
                cns, sns = cn_sb[:, c0:], sn_sb[:, c0:]
                t0_, t1_ = tmp0[:, c0:], tmp1[:, c0:]
                nc.vector.tensor_tensor(t0_, qo, sns, mult)
                nc.vector.tensor_tensor(t1_, qe, sns, mult)
                nc.vector.tensor_tensor(qe, qe, cns, mult)
                nc.vector.tensor_tensor(qo, qo, cns, mult)
                nc.vector.tensor_tensor(qe, qe, t0_, sub_op)
                nc.vector.tensor_tensor(qo, qo, t1_, add_op)

            # chunked scan with PSUM-resident fp32 state
            state_ps = ps_state.tile([P, 2, D], f32, tag="state")
            out_sbs = [
                outp.tile([P, NCH // 2, D], f16, tag=f"out{h}", name=f"out_sb{h}")
                for h in range(2)
            ]
            for s in range(NSUP):
                if s > 0:
                    state_sb = statep.tile([P, 2, D], f16, tag="state_sb")
                    for m in range(2):
                        _copy(
                            nc, STATE_CAST_ENG[m],
                            state_sb[:, m, :], state_ps[:, m, :],
                        )

                # Transposed score blocks for the superchunk's two chunks
                # j0 = 2s, j1 = 2s+1, into one PSUM tile: G_j0 at cols
                # 0:256 (its diag block + the j1 block), G_j1 at 256:384.
                j0 = SUP * s
                g_ps = ps_g.tile([P, 384], f32, tag="g", name="g_ps")
                nc.tensor.matmul(
                    g_ps[:, 0:256], qt_sb[:, 0, ds(j0 * P, P)],
                    qt_sb[:, 0, ds(j0 * P, 256)], start=True, stop=False,
                )
                nc.tensor.matmul(
                    g_ps[:, 0:256], qt_sb[:, 1, ds(j0 * P, P)],
                    qt_sb[:, 1, ds(j0 * P, 256)], start=False, stop=True,
                )
                nc.tensor.matmul(
                    g_ps[:, 256:384], qt_sb[:, 0, ds((j0 + 1) * P, P)],
                    qt_sb[:, 0, ds((j0 + 1) * P, P)], start=True, stop=False,
                )
                nc.tensor.matmul(
                    g_ps[:, 256:384], qt_sb[:, 1, ds((j0 + 1) * P, P)],
                    qt_sb[:, 1, ds((j0 + 1) * P, P)], start=False, stop=True,
                )
                # fp8 pair tile [p, j', i-col]: row 0 = G_j0 (diag-masked
                # then ones), row 1 = [zeros | G_j1 diag-masked] — the
                # leading 128 cols of row 1 read stale PSUM x a zero mask.
                g2 = work.tile([P, 2, SUP * P], f8, tag="g2", name="g2")
                nc.vector.tensor_tensor(g2[:, 0], g_ps[:, 0:256], msk_sb[:, 0], mult)
                nc.vector.tensor_tensor(g2[:, 1], g_ps[:, 128:384], msk_sb[:, 1], mult)

                if s == 0:
                    if bh == 0:
                        # first bh: get super-0's natural-layout qr by
                        # PE-transposing the rope'd qt_sb instead of waiting
                        # on the strict-FIFO DVE queue; rope only chunks
                        # SUP.. on DVE.
                        for ci2 in range(SUP):
                            for m in range(2):
                                t_ps = ps_g.tile([P, P], f16, tag="g", name="t_ps")
                                nc.tensor.transpose(
                                    t_ps[:], qt_sb[:, m, ds(ci2 * P, P)], ident[:]
                                )
                                nc.vector.tensor_copy(qn_sb[:, m, ci2, :], t_ps[:])
                        emit_nat_rope(SUP)
                    else:
                        emit_nat_rope()

                for ci in range(SUP):
                    i = SUP * s + ci
                    # state += qr_c^T v_c (PSUM accumulate), emitted before
                    # the PV matmuls so the superchunk's last state matmul
                    # retires early and the next state cast overlaps the
                    # remaining out work.  Each superchunk's accumulation is
                    # a CLOSED group (stop=True on its last matmul): the
                    # state bank is read (cast) between superchunks.  State
                    # after the last superchunk is never read -> skipped.
                    if 0 < s < NSUP - 1:
                        for m in range(2):
                            for h in range(2):
                                dsl = ds(h * 512, 512)
                                nc.tensor.matmul(
                                    state_ps[:, m, dsl],
                                    qn_sb[:, m, i, :],
                                    v_sb[:, i, dsl],
                                    start=False,
                                    stop=(ci == SUP - 1),
                                    skip_group_check=True,
                                )
                    out_ps = [
                        ps_out.tile([P, 512], f32, tag="outp", name=f"out_ps{h}")
                        for h in range(2)
                    ]
                    if s > 0:
                        # m-outer / h-inner: consecutive matmuls share lhsT
                        for m in range(2):
                            for h in range(2):
                                nc.tensor.matmul(
                                    out_ps[h][:],
                                    qt_sb[:, m, ds(i * P, P)],
                                    state_sb[:, m, ds(h * 512, 512)],
                                    start=(m == 0), stop=False,
                                    skip_group_check=True,
                                )
                    # PV: one fp8 DoubleRow matmul per D-half, contracting
                    # both chunks of the superchunk at 2x rate.
                    for h in range(2):
                        nc.tensor.matmul(
                            out_ps[h][:],
                            g2[:, :, ds(ci * P, P)],
                            v8_sb[:, ds(j0, SUP), ds(h * 512, 512)],
                            start=(s == 0), stop=True,
                            perf_mode=DR,
                            skip_group_check=True,
                        )

                    out_sb = out_sbs[i // (NCH // 2)]
                    for h in range(2):
                        _copy(
                            nc, OUT_EVAC_ENG[i % 2],
                            out_sb[:, i % (NCH // 2), ds(h * 512, 512)],
                            out_ps[h][:],
                        )
                    if ci == SUP - 1:
                        q0 = j0
                        nc.sync.dma_start(
                            out[bh, :, ds(q0, SUP), :],
                            out_sbs[q0 // (NCH // 2)][:, ds(q0 % (NCH // 2), SUP)],
                        )

                if s == 0:
                    for ci2 in range(SUP):
                        for m in range(2):
                            for h in range(2):
                                dsl = ds(h * 512, 512)
                                nc.tensor.matmul(
                                    state_ps[:, m, dsl],
                                    qn_sb[:, m, ci2, :],
                                    v_sb[:, ci2, dsl],
                                    start=(ci2 == 0),
                                    stop=(ci2 == SUP - 1),
                                    skip_group_check=True,
                                )


_BUILT = {}


def _build():
    if "nc" in _BUILT:
        return _BUILT["nc"]
    nc = bacc.Bacc(
        "TRN2", target_bir_lowering=False, debug=False,
        enable_asserts=True, num_devices=NCORES,
    )
    qn = nc.dram_tensor("qn", [HPC, P, 2, NCH, P], f16, kind="ExternalInput")
    qt = nc.dram_tensor("qt", [HPC, 2, P, T], f16, kind="ExternalInput")
    v = nc.dram_tensor("v", [P, NCH, D], f16, kind="ExternalInput")
    v8 = nc.dram_tensor("v8", [P, NCH, D], f8, kind="ExternalInput")
    cn = nc.dram_tensor("cn", [P, NCH, P], f16, kind="ExternalInput")
    sn = nc.dram_tensor("sn", [P, NCH, P], f16, kind="ExternalInput")
    ct = nc.dram_tensor("ct", [P, T], f16, kind="ExternalInput")
    st = nc.dram_tensor("st", [P, T], f16, kind="ExternalInput")
    mskT = nc.dram_tensor("mskT", [P, 2, SUP * P], f16, kind="ExternalInput")
    out = nc.dram_tensor("out", [HPC, P, NCH, D], f16, kind="ExternalOutput")
    with tile.TileContext(nc) as tc:
        _emit_body(nc, tc, qn, qt, v, v8, cn, sn, ct, st, mskT, out)
    nc.compile()
    _BUILT["nc"] = nc
    return nc


def _host_prep(Q_raw, V_raw):
    """Shard + precompute device inputs (fp16/fp8, partition-major)."""
    Q = np.asarray(Q_raw, dtype=np.float32)
    V = np.asarray(V_raw, dtype=np.float32)

    # rope tables, matching reference._get_freqs / _rope in float32
    t = np.arange(N, dtype=np.float32)
    q = np.floor(t / 2.0) * 2.0
    freqs = (1.0 / (THETA ** (q / np.float32(N))) / np.float32(TWO_PI)).astype(
        np.float32
    )
    phases = np.arange(T, dtype=np.float32)[:, None] * freqs[None, :]
    ph = (phases % 1.0) * np.float32(TWO_PI)
    # freqs are equal within each (even, odd) pair -> keep only even columns
    cosf = np.cos(ph[:, 0::2]).astype(f16_np)        # [T, 128]
    sinf = np.sin(ph[:, 0::2]).astype(f16_np)
    # natural tables [P, NCH, P]: (p, c, k) = table[c*128+p, k]
    cn = np.ascontiguousarray(cosf.reshape(NCH, P, P).transpose(1, 0, 2))
    sn = np.ascontiguousarray(sinf.reshape(NCH, P, P).transpose(1, 0, 2))
    # transposed tables [P, T]: (k, t)
    ct = np.ascontiguousarray(cosf.T)
    st = np.ascontiguousarray(sinf.T)
    # pair-tile masks [P, 2, 2P]: row 0 = [strict-triu | ones] (G_j0: diag
    # block then the full j1 block), row 1 = [zeros | strict-triu] (G_j1)
    mskT = np.zeros((P, 2, SUP * P), np.float32)
    mskT[:, 0, :P] = np.triu(np.ones((P, P), np.float32), k=1)
    mskT[:, 0, P:] = 1.0
    mskT[:, 1, P:] = np.triu(np.ones((P, P), np.float32), k=1)
    mskT = mskT.astype(f16_np)

    # deinterleave pairs: planes (evens, odds), cast fp16
    Qd = np.stack([Q[..., 0::2], Q[..., 1::2]], axis=2).astype(f16_np)
    # Qd: [B, NH, 2, T, 128]
    # natural layout  [b,h][p, half, c, k] = Qd[b, h, half, c*128+p, k]
    Qn = np.ascontiguousarray(
        Qd.reshape(B, NH, 2, NCH, P, P).transpose(0, 1, 4, 2, 3, 5)
    )  # [B, NH, P, 2, NCH, P]
    # transposed layout [b,h][half, k, t] = Qd[b, h, half, t, k]
    Qt = np.ascontiguousarray(Qd.transpose(0, 1, 2, 4, 3))  # [B, NH, 2, 128, T]

    V16 = V.astype(f16_np)
    # v layout [P, NCH, D]: (p, c, d) = V[c*128+p, d]
    Vp = np.ascontiguousarray(V16.reshape(B, NCH, P, D).transpose(0, 2, 1, 3))
    V8p = Vp.astype(f8_np)

    in_maps = []
    for core in range(NCORES):
        b = core // (NCORES // B)
        hs = (core % (NCORES // B)) * HPC
        in_maps.append(
            {
                "qn": np.ascontiguousarray(Qn[b, hs : hs + HPC]),
                "qt": np.ascontiguousarray(Qt[b, hs : hs + HPC]),
                "v": Vp[b],
                "v8": V8p[b],
                "cn": cn,
                "sn": sn,
                "ct": ct,
                "st": st,
                "mskT": mskT,
            }
        )
    return in_maps


def _run(inputs, trace=False, **kw):
    nc = _build()
    in_maps = _host_prep(inputs["Q_raw"], inputs["V_raw"])
    res = run_bass_kernel_spmd(nc, in_maps, list(range(NCORES)), trace=trace, **kw)
    out = np.empty((B, NH, T, D), dtype=np.float32)
    for core in range(NCORES):
        b = core // (NCORES // B)
        hs = (core % (NCORES // B)) * HPC
        # device out: [HPC, P, NCH, D] partition-major -> [HPC, T, D]
        o = res.results[core]["out"].astype(np.float32)
        out[b, hs : hs + HPC] = o.transpose(0, 2, 1, 3).reshape(HPC, T, D)
    return out, res


def kernel(**inputs):
    out, _ = _run(inputs)
    return out


# revision 11
# speedup vs baseline: 1.0240x; 1.0240x over previous
"""Trainium2 Bass kernel for BDH recurrent (chunked linear) attention.

Problem shapes (hardcoded): Q_raw [2,16,2048,256] f32, V_raw [2,2048,1024] f32,
out [2,16,2048,1024] f32.  8 NeuronCores, data+head parallel: each core owns
4 (batch, head) pairs; V is shared across the 4 heads of a core's batch.

Math (reference semantics), per (b,h), chunks of 128:
  QR = rope(Q); KR = QR
  out_c = q_c @ state_{<c} + (q_c q_c^T  * strict_tril) v_c
  state += q_c^T v_c

v2 design:
  * fp16 everywhere 16-bit (same PE/DVE speed as bf16, ~8x the accuracy),
    which buys error budget for:
  * fp8(e4m3) DoubleRow PV matmuls: per superchunk of SUP=2 chunks, the
    transposed score blocks G_j (j = the 2 chunks) are evacuated into one
    [128, 2, 256] fp8 pair tile; the PV for chunk i is then ONE DoubleRow
    matmul per D-half contracting 256 rows (both j chunks) at 2x rate.
    The pair row of the later chunk j1 is zero in its leading 128 cols
    (evac reads 128 stale PSUM cols x a zero mask), preserving causality.
  * SUP=2 minimizes total matmul work (intra-superchunk O(S^2) term vs
    the per-chunk state/inter term).
  * state/inter matmuls stay fp16 (fp8 there fails the 2e-2 gate).
  * PSUM-resident fp32 state, cast to fp16 SBUF between superchunks.
  * evacuation/cast copies spread over scalar+gpsimd (knobs below); DVE
    keeps rope + masked G evacuation.

RoPE is computed twice, in the natural [t, n] layout (for the state update's
lhsT) and in the transposed [n, t] layout (for the G lhsT and the inter
lhsT) — the host supplies Q in both layouts (pair-deinterleaved:
(evens | odds), so the rotation is plane-wise multiply/adds with no
interleave shuffles), and rope runs in-place on the loaded tiles.
All DRAM layouts are partition-major so every DMA is 128 contiguous
descriptors; the output is written partition-major and un-permuted on host.
"""

import numpy as np
import ml_dtypes

import concourse.mybir as mybir
import concourse.tile as tile
from concourse import bacc
from concourse.bass import ds
from concourse.bass_utils import run_bass_kernel_spmd

B, NH, T, N, D = 2, 16, 2048, 256, 1024
P = 128          # partition / chunk size
NCH = T // P     # 16 chunks
SUP = 2          # chunks per superchunk
NSUP = NCH // SUP
HPC = 4          # (b,h) pairs per core
NCORES = 8
THETA = 2.0 ** 16
TWO_PI = 2.0 * np.pi

f16 = mybir.dt.float16
f8 = mybir.dt.float8e4
f32 = mybir.dt.float32
f16_np = np.float16
f8_np = ml_dtypes.float8_e4m3  # TRN-style e4m3 (max normal 240)

mult = mybir.AluOpType.mult
add_op = mybir.AluOpType.add
sub_op = mybir.AluOpType.subtract
DR = mybir.MatmulPerfMode.DoubleRow

# engine assignment knobs (tuned from profiles).
# NB: gpsimd has NO PSUM port (walrus birverifier rejects it) and its
# tensor_tensor contends with DVE's shared SBUF port -> PSUM evacuation
# copies go on scalar/vector only; DVE keeps rope + masked G evac.
STATE_CAST_ENG = ("s", "s")   # by m-plane
OUT_EVAC_ENG = ("s", "s", "v")  # by chunk index mod 3


def _eng(nc, c):
    return {"g": nc.gpsimd, "v": nc.vector, "s": nc.scalar}[c]


def _copy(nc, c, out, in_):
    if c == "s":
        nc.scalar.copy(out, in_)
    else:
        _eng(nc, c).tensor_copy(out, in_)


def _emit_body(nc, tc, qn, qt, v, v8, cn, sn, ct, st, mskT, out):
    """Tile program for one core: 4 (b,h) pairs, full scan each."""
    with (
        tc.tile_pool(name="const", bufs=1) as constp,
        tc.tile_pool(name="qpool", bufs=2) as qpool,
        tc.tile_pool(name="work", bufs=4) as work,
        tc.tile_pool(name="tmppool", bufs=1) as tmpp,
        tc.tile_pool(name="outbuf", bufs=1) as outp,
        tc.tile_pool(name="statesb", bufs=2) as statep,
        tc.tile_pool(name="ps_state", bufs=1, space="PSUM") as ps_state,
        tc.tile_pool(name="ps_out", bufs=3, space="PSUM") as ps_out,
        tc.tile_pool(name="ps_g", bufs=1, space="PSUM") as ps_g,
    ):
        # resident constants (all DRAM layouts partition-major/contiguous).
        # Load order matters for the startup ramp: the first-superchunk
        # slices of the transposed-rope tables gate the first rope ops ->
        # first G matmuls, so they go first; v8's first slice feeds the
        # first PV.
        S0 = SUP * P  # first-slice width in the t dimension
        ct_sb = constp.tile([P, T], f16)
        st_sb = constp.tile([P, T], f16)
        nc.sync.dma_start(ct_sb[:, ds(0, S0)], ct[:, ds(0, S0)])
        nc.sync.dma_start(st_sb[:, ds(0, S0)], st[:, ds(0, S0)])
        msk_sb = constp.tile([P, 2, SUP * P], f16)
        nc.sync.dma_start(msk_sb[:], mskT[:, :, :])
        v8_sb = constp.tile([P, NCH, D], f8)
        nc.sync.dma_start(v8_sb[:, :SUP], v8[:, :SUP, :])
        v_sb = constp.tile([P, NCH, D], f16)
        nc.sync.dma_start(v_sb[:, :SUP], v[:, :SUP, :])
        nc.sync.dma_start(ct_sb[:, ds(S0, T - S0)], ct[:, ds(S0, T - S0)])
        nc.sync.dma_start(st_sb[:, ds(S0, T - S0)], st[:, ds(S0, T - S0)])
        cn_sb = constp.tile([P, NCH, P], f16)
        nc.sync.dma_start(cn_sb[:], cn[:, :, :])
        sn_sb = constp.tile([P, NCH, P], f16)
        nc.sync.dma_start(sn_sb[:], sn[:, :, :])
        nc.sync.dma_start(v8_sb[:, SUP:], v8[:, SUP:, :])
        nc.sync.dma_start(v_sb[:, SUP : NCH // 2], v[:, SUP : NCH // 2, :])
        nc.sync.dma_start(v_sb[:, NCH // 2 :], v[:, NCH // 2 :, :])

        for bh in range(HPC):
            # q tiles; rope runs in-place so qt_sb becomes qrT and qn_sb
            # becomes qr (natural layout).
            qt_sb = qpool.tile([P, 2, T], f16, tag="qt")
            if bh == 0:
                for m in range(2):
                    nc.scalar.dma_start(qt_sb[:, m, ds(0, S0)], qt[bh, m, :, ds(0, S0)])
                for m in range(2):
                    nc.scalar.dma_start(
                        qt_sb[:, m, ds(S0, T - S0)], qt[bh, m, :, ds(S0, T - S0)]
                    )
            else:
                nc.scalar.dma_start(qt_sb[:, 0], qt[bh, 0])
                nc.scalar.dma_start(qt_sb[:, 1], qt[bh, 1])
            qn_sb = qpool.tile([P, 2, NCH, P], f16, tag="qn")
            nc.scalar.dma_start(qn_sb[:], qn[bh])

            # RoPE, transposed layout [n', t], in-place on qt_sb — emitted
            # FIRST: it gates the G matmuls, and the DVE queue is strict
            # FIFO.  qr_e = qe*c - qo*s ; qr_o = qo*c + qe*s, with the tmp
            # products computed before the in-place overwrite of qe/qo.
            tmp2 = tmpp.tile([P, T], f16, tag="ropetmp2")
            tmp3 = tmpp.tile([P, T], f16, tag="ropetmp3")

            def emit_t_rope(c0, w):
                sl = ds(c0, w)
                qte, qto = qt_sb[:, 0, sl], qt_sb[:, 1, sl]
                t2, t3 = tmp2[:, sl], tmp3[:, sl]
                nc.vector.tensor_tensor(t2, qto, st_sb[:, sl], mult)
                nc.vector.tensor_tensor(t3, qte, st_sb[:, sl], mult)
                nc.vector.tensor_tensor(qte, qte, ct_sb[:, sl], mult)
                nc.vector.tensor_tensor(qto, qto, ct_sb[:, sl], mult)
                nc.vector.tensor_tensor(qte, qte, t2, sub_op)
                nc.vector.tensor_tensor(qto, qto, t3, add_op)

            # RoPE, natural layout, in-place on qn_sb (planes (evens|odds)).
            # Emitted lazily (at s==0, after the first G evacuations): qr
            # only gates the state-update matmuls.
            def emit_nat_rope(c0, ncs):
                tmp0 = tmpp.tile([P, NCH, P], f16, tag="ropetmp0")
                tmp1 = tmpp.tile([P, NCH, P], f16, tag="ropetmp1")
                sl = ds(c0, ncs)
                qe, qo = qn_sb[:, 0, sl], qn_sb[:, 1, sl]
                cns, sns = cn_sb[:, sl], sn_sb[:, sl]
                t0_, t1_ = tmp0[:, sl], tmp1[:, sl]
                nc.vector.tensor_tensor(t0_, qo, sns, mult)
                nc.vector.tensor_tensor(t1_, qe, sns, mult)
                nc.vector.tensor_tensor(qe, qe, cns, mult)
                nc.vector.tensor_tensor(qo, qo, cns, mult)
                nc.vector.tensor_tensor(qe, qe, t0_, sub_op)
                nc.vector.tensor_tensor(qo, qo, t1_, add_op)

            if bh == 0:
                # bh0 cold-start: rope only what gates the first matmuls
                # (transposed cols of superchunk 0, natural chunks 0..SUP);
                # the rest is emitted inside the scan (after each G evac)
                # so the strict-FIFO DVE queue never blocks the PE.
                emit_t_rope(0, S0)
                emit_nat_rope(0, SUP)
            else:
                emit_t_rope(0, T)

            # chunked scan with PSUM-resident fp32 state
            state_ps = ps_state.tile([P, 2, D], f32, tag="state")
            out_sbs = [
                outp.tile([P, NCH // 2, D], f16, tag=f"out{h}", name=f"out_sb{h}")
                for h in range(2)
            ]
            for s in range(NSUP):
                if s > 0:
                    state_sb = statep.tile([P, 2, D], f16, tag="state_sb")
                    for m in range(2):
                        _copy(
                            nc, STATE_CAST_ENG[m],
                            state_sb[:, m, :], state_ps[:, m, :],
                        )

                # Transposed score blocks for the superchunk's two chunks
                # j0 = 2s, j1 = 2s+1, into one PSUM tile: G_j0 at cols
                # 0:256 (its diag block + the j1 block), G_j1 at 256:384.
                j0 = SUP * s
                g_ps = ps_g.tile([P, 384], f32, tag="g", name="g_ps")
                nc.tensor.matmul(
                    g_ps[:, 0:256], qt_sb[:, 0, ds(j0 * P, P)],
                    qt_sb[:, 0, ds(j0 * P, 256)], start=True, stop=False,
                )
                nc.tensor.matmul(
                    g_ps[:, 0:256], qt_sb[:, 1, ds(j0 * P, P)],
                    qt_sb[:, 1, ds(j0 * P, 256)], start=False, stop=True,
                )
                nc.tensor.matmul(
                    g_ps[:, 256:384], qt_sb[:, 0, ds((j0 + 1) * P, P)],
                    qt_sb[:, 0, ds((j0 + 1) * P, P)], start=True, stop=False,
                )
                nc.tensor.matmul(
                    g_ps[:, 256:384], qt_sb[:, 1, ds((j0 + 1) * P, P)],
                    qt_sb[:, 1, ds((j0 + 1) * P, P)], start=False, stop=True,
                )
                # fp8 pair tile [p, j', i-col]: row 0 = G_j0 (diag-masked
                # then ones), row 1 = [zeros | G_j1 diag-masked] — the
                # leading 128 cols of row 1 read stale PSUM x a zero mask.
                g2 = work.tile([P, 2, SUP * P], f8, tag="g2", name="g2")
                nc.vector.tensor_tensor(g2[:, 0], g_ps[:, 0:256], msk_sb[:, 0], mult)
                nc.vector.tensor_tensor(g2[:, 1], g_ps[:, 128:384], msk_sb[:, 1], mult)

                if bh == 0:
                    # staged rope emission behind each early G evac
                    if s == 0:
                        emit_t_rope(S0, 3 * S0)
                        emit_nat_rope(SUP, NCH // 2 - SUP)
                    elif s == 1:
                        emit_t_rope(4 * S0, T - 4 * S0)
                        emit_nat_rope(NCH // 2, NCH // 2)
                elif s == 0:
                    emit_nat_rope(0, NCH)

                for ci in range(SUP):
                    i = SUP * s + ci
                    # state += qr_c^T v_c (PSUM accumulate), emitted before
                    # the PV matmuls so the superchunk's last state matmul
                    # retires early and the next state cast overlaps the
                    # remaining out work.  Each superchunk's accumulation is
                    # a CLOSED group (stop=True on its last matmul): the
                    # state bank is read (cast) between superchunks.  State
                    # after the last superchunk is never read -> skipped.
                    if 0 < s < NSUP - 1:
                        for m in range(2):
                            for h in range(2):
                                dsl = ds(h * 512, 512)
                                nc.tensor.matmul(
                                    state_ps[:, m, dsl],
                                    qn_sb[:, m, i, :],
                                    v_sb[:, i, dsl],
                                    start=False,
                                    stop=(ci == SUP - 1),
                                    skip_group_check=True,
                                )
                    out_ps = [
                        ps_out.tile([P, 512], f32, tag="outp", name=f"out_ps{h}")
                        for h in range(2)
                    ]
                    if s > 0:
                        # m-outer / h-inner: consecutive matmuls share lhsT
                        for m in range(2):
                            for h in range(2):
                                nc.tensor.matmul(
                                    out_ps[h][:],
                                    qt_sb[:, m, ds(i * P, P)],
                                    state_sb[:, m, ds(h * 512, 512)],
                                    start=(m == 0), stop=False,
                                    skip_group_check=True,
                                )
                    # PV: one fp8 DoubleRow matmul per D-half, contracting
                    # both chunks of the superchunk at 2x rate.
                    for h in range(2):
                        nc.tensor.matmul(
                            out_ps[h][:],
                            g2[:, :, ds(ci * P, P)],
                            v8_sb[:, ds(j0, SUP), ds(h * 512, 512)],
                            start=(s == 0), stop=True,
                            perf_mode=DR,
                            skip_group_check=True,
                        )

                    out_sb = out_sbs[i // (NCH // 2)]
                    for h in range(2):
                        _copy(
                            nc, OUT_EVAC_ENG[i % len(OUT_EVAC_ENG)],
                            out_sb[:, i % (NCH // 2), ds(h * 512, 512)],
                            out_ps[h][:],
                        )
                    if s == NSUP - 1 and bh == HPC - 1:
                        # drain tail: per-chunk DMA for the last superchunk
                        nc.sync.dma_start(
                            out[bh, :, ds(i, 1), :],
                            out_sbs[i // (NCH // 2)][:, ds(i % (NCH // 2), 1)],
                        )
                    elif ci == SUP - 1:
                        q0 = j0
                        nc.sync.dma_start(
                            out[bh, :, ds(q0, SUP), :],
                            out_sbs[q0 // (NCH // 2)][:, ds(q0 % (NCH // 2), SUP)],
                        )

                if s == 0:
                    for ci2 in range(SUP):
                        for m in range(2):
                            for h in range(2):
                                dsl = ds(h * 512, 512)
                                nc.tensor.matmul(
                                    state_ps[:, m, dsl],
                                    qn_sb[:, m, ci2, :],
                                    v_sb[:, ci2, dsl],
                                    start=(ci2 == 0),
                                    stop=(ci2 == SUP - 1),
                                    skip_group_check=True,
                                )


_BUILT = {}


def _build():
    if "nc" in _BUILT:
        return _BUILT["nc"]
    nc = bacc.Bacc(
        "TRN2", target_bir_lowering=False, debug=False,
        enable_asserts=True, num_devices=NCORES,
    )
    qn = nc.dram_tensor("qn", [HPC, P, 2, NCH, P], f16, kind="ExternalInput")
    qt = nc.dram_tensor("qt", [HPC, 2, P, T], f16, kind="ExternalInput")
    v = nc.dram_tensor("v", [P, NCH, D], f16, kind="ExternalInput")
    v8 = nc.dram_tensor("v8", [P, NCH, D], f8, kind="ExternalInput")
    cn = nc.dram_tensor("cn", [P, NCH, P], f16, kind="ExternalInput")
    sn = nc.dram_tensor("sn", [P, NCH, P], f16, kind="ExternalInput")
    ct = nc.dram_tensor("ct", [P, T], f16, kind="ExternalInput")
    st = nc.dram_tensor("st", [P, T], f16, kind="ExternalInput")
    mskT = nc.dram_tensor("mskT", [P, 2, SUP * P], f16, kind="ExternalInput")
    out = nc.dram_tensor("out", [HPC, P, NCH, D], f16, kind="ExternalOutput")
    with tile.TileContext(nc) as tc:
        _emit_body(nc, tc, qn, qt, v, v8, cn, sn, ct, st, mskT, out)
    nc.compile()
    _BUILT["nc"] = nc
    return nc


def _host_prep(Q_raw, V_raw):
    """Shard + precompute device inputs (fp16/fp8, partition-major)."""
    Q = np.asarray(Q_raw, dtype=np.float32)
    V = np.asarray(V_raw, dtype=np.float32)

    # rope tables, matching reference._get_freqs / _rope in float32
    t = np.arange(N, dtype=np.float32)
    q = np.floor(t / 2.0) * 2.0
    freqs = (1.0 / (THETA ** (q / np.float32(N))) / np.float32(TWO_PI)).astype(
        np.float32
    )
    phases = np.arange(T, dtype=np.float32)[:, None] * freqs[None, :]
    ph = (phases % 1.0) * np.float32(TWO_PI)
    # freqs are equal within each (even, odd) pair -> keep only even columns
    cosf = np.cos(ph[:, 0::2]).astype(f16_np)        # [T, 128]
    sinf = np.sin(ph[:, 0::2]).astype(f16_np)
    # natural tables [P, NCH, P]: (p, c, k) = table[c*128+p, k]
    cn = np.ascontiguousarray(cosf.reshape(NCH, P, P).transpose(1, 0, 2))
    sn = np.ascontiguousarray(sinf.reshape(NCH, P, P).transpose(1, 0, 2))
    # transposed tables [P, T]: (k, t)
    ct = np.ascontiguousarray(cosf.T)
    st = np.ascontiguousarray(sinf.T)
    # pair-tile masks [P, 2, 2P]: row 0 = [strict-triu | ones] (G_j0: diag
    # block then the full j1 block), row 1 = [zeros | strict-triu] (G_j1)
    mskT = np.zeros((P, 2, SUP * P), np.float32)
    mskT[:, 0, :P] = np.triu(np.ones((P, P), np.float32), k=1)
    mskT[:, 0, P:] = 1.0
    mskT[:, 1, P:] = np.triu(np.ones((P, P), np.float32), k=1)
    mskT = mskT.astype(f16_np)

    # deinterleave pairs: planes (evens, odds), cast fp16
    Qd = np.stack([Q[..., 0::2], Q[..., 1::2]], axis=2).astype(f16_np)
    # Qd: [B, NH, 2, T, 128]
    # natural layout  [b,h][p, half, c, k] = Qd[b, h, half, c*128+p, k]
    Qn = np.ascontiguousarray(
        Qd.reshape(B, NH, 2, NCH, P, P).transpose(0, 1, 4, 2, 3, 5)
    )  # [B, NH, P, 2, NCH, P]
    # transposed layout [b,h][half, k, t] = Qd[b, h, half, t, k]
    Qt = np.ascontiguousarray(Qd.transpose(0, 1, 2, 4, 3))  # [B, NH, 2, 128, T]

    V16 = V.astype(f16_np)
    # v layout [P, NCH, D]: (p, c, d) = V[c*128+p, d]
    Vp = np.ascontiguousarray(V16.reshape(B, NCH, P, D).transpose(0, 2, 1, 3))
    V8p = Vp.astype(f8_np)

    in_maps = []
    for core in range(NCORES):
        b = core // (NCORES // B)
        hs = (core % (NCORES // B)) * HPC
        in_maps.append(
            {
                "qn": np.ascontiguousarray(Qn[b, hs : hs + HPC]),
                "qt": np.ascontiguousarray(Qt[b, hs : hs + HPC]),
                "v": Vp[b],
                "v8": V8p[b],
                "cn": cn,
                "sn": sn,
                "ct": ct,
                "st": st,
                "mskT": mskT,
            }
        )
    return in_maps


def _run(inputs, trace=False, **kw):
    nc = _build()
    in_maps = _host_prep(inputs["Q_raw"], inputs["V_raw"])
    res = run_bass_kernel_spmd(nc, in_maps, list(range(NCORES)), trace=trace, **kw)
    out = np.empty((B, NH, T, D), dtype=np.float32)
    for core in range(NCORES):
        b = core // (NCORES // B)
        hs = (core % (NCORES // B)) * HPC
        # device out: [HPC, P, NCH, D] partition-major -> [HPC, T, D]
        o = res.results[core]["out"].astype(np.float32)
        out[b, hs : hs + HPC] = o.transpose(0, 2, 1, 3).reshape(HPC, T, D)
    return out, res


def kernel(**inputs):
    out, _ = _run(inputs)
    return out
